# revision 15
# baseline (speedup 1.0000x reference)
"""Distributed Trainium2 kernel for the sparse-attention nn.Module.

Reference computation (per full batch B=32):
    m  = s @ W_phi + b_phi                    [B, Tq, H*P]
    n  = h @ W_psi                            [B, Tk, P]
    e  = einsum('bqhp,bkp->hbqk', m.resh, n)  [H, B, Tq, Tk]
    emax = global per-head max of e           (max over B, Tq, Tk!)
    exps = exp(e - emax) * mask
    a  = exps / (sum_k exps + EPS)            [H, B, Tq, Tk]
    ctx = einsum('hbqk,bkd->bqhd', a, h)      -> [B, Tq, H*LV]
    c  = ctx @ W_red + b_red                  [B, Tq, LV]
    returns (c, a.transpose(1,0,2,3))

Sharding: data-parallel over batch across 8 NeuronCores (4 batches/core);
the only cross-core dependency is the global per-head max -> tiny
AllReduce(max) of 4 floats mid-kernel.

Precision: the e einsum accumulates in fp32 PSUM from fp32 mT/nT
tiles; the s/h projections and the attention-application side run bf16
(fp32 PSUM accumulation everywhere).  Exp amplifies score errors
(rowsums ~1e-4 vs EPS=1e-5), so full-bf16 scores would be risky; this
mix measures ~8.7e-3 rel err on HW vs the 2e-2 gate.  K_SBF=0 /
K_AT_MODE=pe env flags fall back to more conservative variants.

Layout strategy (per core, B_loc=4, Q=1024 query rows):
  - mT[ph, q] = W_phi.T @ s.T; s.T via PE-transpose.
  - nT[p, k]  = W_psi.T @ h.T; h.T via PE-transpose.
  - e[q, k] = matmul(lhsT=mT chunks, rhs=nT) fp32 -> PSUM; evacuated by a
    DVE scalar_tensor_tensor that also folds the mask in additively
    (e + (mask-1)*BIG), and spilled to DRAM (16 MiB of scores don't fit
    in SBUF).  Per-row max comes from a DVE reduce directly on the PSUM
    tile (raw e, pre-mask, matching the reference).
  - softmax: ACT exp with per-partition bias (-emax) and fused row-sum
    accumulation; DVE reciprocal + scale.
  - a (f32) is DMA'd out; a bf16 copy is transposed k-major via the DMA
    xbar (one 3D-out call per [128,1024] tile).
  - ctxT[d, q] = matmul(lhsT=h(bf16), rhs=aT(bf16)); two heads packed in
    the moving operand (N=512).
  - c[q, dout] = matmul(lhsT=ctxT chunks, rhs=W_red(bf16)) + b_red.

NB: tensor_tensor_reduce is avoided entirely -- it crashes the NC
(NRT_EXEC_UNIT_UNRECOVERABLE) on this runtime, from both PSUM and SBUF.
"""

import os as _os
import numpy as np
from contextlib import ExitStack

import concourse.bass as bass
import concourse.mybir as mybir
import concourse.tile as tile
from concourse import bacc
from concourse.bass_utils import run_bass_kernel_spmd
from concourse.masks import make_identity

F32 = mybir.dt.float32
F32R = mybir.dt.float32r
BF16 = mybir.dt.bfloat16
AX = mybir.AxisListType
ALU = mybir.AluOpType
ACTF = mybir.ActivationFunctionType

B_FULL, TQ, TK = 32, 256, 1024
SV, LV, P, H = 1024, 512, 256, 4
NCORES = 8
BL = B_FULL // NCORES      # local batches per core
Q = BL * TQ                # local query rows (1024)
EPS = 1e-5
NEG_INF = -1e30
MASK_BIG = 1e4             # additive mask: e + (mask-1)*MASK_BIG

AT_MODE = _os.environ.get("K_AT_MODE", "xbar3d")   # "xbar3d" | "pe"
USE_F32R = _os.environ.get("K_F32R", "0") == "1"   # float32r e-chain matmuls
TRACE_SIM = _os.environ.get("K_TRACE_SIM", "0") == "1"
USE_SBF = _os.environ.get("K_SBF", "1") == "1"   # bf16 s->m chain (xbar sT)


FCH = F32R if USE_F32R else F32   # dtype of score-chain matmul operands


def _r(ap):
    return ap


def _ap(handle_ap, ap_list, offset=None):
    """Build a raw AP (for broadcast / strided access) on a tensor."""
    return bass.AP(
        tensor=handle_ap.tensor,
        offset=handle_ap.offset if offset is None else offset,
        ap=ap_list,
    )


def build_kernel() -> bass.Bass:
    nc = bacc.Bacc()

    s_d = nc.declare_dram_parameter("s", [BL, TQ, SV], F32, isOutput=False)
    h_d = nc.declare_dram_parameter("h", [BL, TK, LV], F32, isOutput=False)
    mask_d = nc.declare_dram_parameter("len_mask", [Q, TK], F32, isOutput=False)
    wphi_d = nc.declare_dram_parameter("W_phi", [SV, P * H], F32, isOutput=False)
    bphi_d = nc.declare_dram_parameter("b_phi", [P * H], F32, isOutput=False)
    wpsi_d = nc.declare_dram_parameter("W_psi", [LV, P], F32, isOutput=False)
    wred_d = nc.declare_dram_parameter("W_red", [LV * H, LV], F32, isOutput=False)
    bred_d = nc.declare_dram_parameter("b_red", [LV], F32, isOutput=False)
    c_d = nc.declare_dram_parameter("c_out", [Q, LV], F32, isOutput=True)
    a_d = nc.declare_dram_parameter("a_out", [BL, H, TQ, TK], F32, isOutput=True)

    s_flat = s_d[:].rearrange("b q v -> (b q) v")      # [1024, 1024]

    e_spill = nc.dram_tensor("e_spill", [BL, H, 2, 128, TK], F32)
    cc_in = nc.dram_tensor("cc_in", [H], F32)
    cc_out = nc.dram_tensor("cc_out", [H], F32, addr_space="Shared")

    with ExitStack() as top:
        tc = top.enter_context(tile.TileContext(nc, trace_sim=TRACE_SIM))

        const = top.enter_context(tc.tile_pool(name="const", bufs=1))
        ident = const.tile([128, 128], F32, tag="ident", name="ident")
        make_identity(nc, ident)
        ident_bf = const.tile([128, 128], BF16, tag="identbf", name="identbf")
        nc.vector.tensor_copy(out=ident_bf, in_=ident)
        # b_phi laid out chunk-major: bphi_sb[p, i] = b_phi[i*128 + p].
        # Load as [8,128] rows (contiguous) then PE-transpose: a direct
        # [[1,128],[128,8]] DMA is 4-byte-strided and costs ~34us.
        bphi_sb = const.tile([128, 8], F32, tag="bphi", name="bphi")
        bphi_row = const.tile([128, 128], F32, tag="bphirow", name="bphirow")
        nc.gpsimd.dma_start(out=bphi_row[:8, :],
                            in_=_ap(bphi_d[:], [[128, 8], [1, 128]]))
        locmax = const.tile([128, H], F32, tag="locmax", name="locmax")
        nc.vector.memset(locmax, NEG_INF)
        gmax_col = const.tile([128, 1], F32, tag="gmaxcol", name="gmaxcol")
        emax_b = const.tile([128, H], F32, tag="emaxb", name="emaxb")
        negmax = const.tile([128, H], F32, tag="negmax", name="negmax")

        # W_red tiles allocated up front; their cast-DMAs are issued after
        # the s/W_phi casts (otherwise they either head the Pool queue and
        # delay the first matmuls, or sit at the barrier stalling a-stores)
        wred_p = top.enter_context(tc.tile_pool(name="wred", bufs=1))
        wred = [wred_p.tile([128, LV], BF16, tag=f"wred{ch}",
                            name=f"wred{ch}") for ch in range(16)]
        # hbf persists into phase C
        hbf_p = top.enter_context(tc.tile_pool(name="hbf", bufs=1))
        hbf = [hbf_p.tile([128, 8, LV], BF16, tag=f"hbf{b}", name=f"hbf{b}")
               for b in range(BL)]

        # ---------------- Phase A ----------------
        with ExitStack() as pha:
            psum_tr = pha.enter_context(
                tc.tile_pool(name="psum_tr", bufs=2, space="PSUM"))
            psum_mm = pha.enter_context(
                tc.tile_pool(name="psum_mm", bufs=2, space="PSUM"))
            psum_e = pha.enter_context(
                tc.tile_pool(name="psum_e", bufs=2, space="PSUM"))
            ps_b = psum_tr.tile([128, 128], F32, tag="pstr", name="pstr")
            nc.tensor.transpose(ps_b[:, :8], bphi_row[:8, :], ident[:8, :8])
            nc.scalar.copy(out=bphi_sb, in_=ps_b[:, :8])
            esb_p = pha.enter_context(tc.tile_pool(name="esb", bufs=4))
            lm_p = pha.enter_context(tc.tile_pool(name="lm", bufs=4))
            mask_p = pha.enter_context(tc.tile_pool(name="maskp", bufs=2))
            mb_p = pha.enter_context(tc.tile_pool(name="mbp", bufs=3))
            wpsi_p = pha.enter_context(tc.tile_pool(name="wpsi", bufs=1))
            wpsi = []
            for dj in range(4):
                t = wpsi_p.tile([128, P], BF16, tag=f"wpsi{dj}",
                                name=f"wpsi{dj}")
                nc.gpsimd.dma_start(
                    out=t, in_=wpsi_d[dj * 128:(dj + 1) * 128, :])
                wpsi.append(t)
            # mT persists through e (phase A only)
            mT_p = pha.enter_context(tc.tile_pool(name="mT", bufs=1))
            mT = [mT_p.tile([128, Q], FCH, tag=f"mT{i}", name=f"mT{i}")
                  for i in range(8)]

            # --- A1/A2: W_phi, s.T, mT = W_phi.T @ s.T (+ b_phi) ---
            with ExitStack() as ph2:
                sdt = BF16 if USE_SBF else FCH
                wphi_p = ph2.enter_context(tc.tile_pool(name="wphi", bufs=1))
                wphi = []
                sin_p = ph2.enter_context(tc.tile_pool(name="sin", bufs=3))
                if USE_SBF:
                    # s -> bf16 cast-DMA, transposed via the DMA xbar.
                    # W_phi casts are interleaved with the s casts so the
                    # first mT matmul isn't queued behind all 8 of them.
                    sT_p = ph2.enter_context(tc.tile_pool(name="sTb", bufs=1))
                    sT_bf = sT_p.tile([128, 8, Q], BF16, tag="sTb",
                                      name="sTb")
                    for qt in range(8):
                        s_in = sin_p.tile([128, SV], BF16, tag="sin",
                                          name="sin")
                        nc.gpsimd.dma_start(
                            out=s_in, in_=s_flat[qt * 128:(qt + 1) * 128, :])
                        nc.sync.dma_start(
                            out=sT_bf[:, :, qt * 128:(qt + 1) * 128],
                            in_=s_in, transpose=True)
                        t = wphi_p.tile([128, P * H], sdt, tag=f"wphi{qt}",
                                        name=f"wphi{qt}")
                        nc.gpsimd.dma_start(
                            out=t, in_=wphi_d[qt * 128:(qt + 1) * 128, :])
                        wphi.append(t)
                    srhs = lambda g, nh: sT_bf[:, g, nh * 512:(nh + 1) * 512]
                else:
                    for g in range(8):
                        t = wphi_p.tile([128, P * H], sdt, tag=f"wphi{g}",
                                        name=f"wphi{g}")
                        nc.sync.dma_start(
                            out=t.bitcast(F32),
                            in_=wphi_d[g * 128:(g + 1) * 128, :])
                        wphi.append(t)
                    sT_p = ph2.enter_context(tc.tile_pool(name="sT", bufs=1))
                    sT = [sT_p.tile([128, Q], FCH, tag=f"sT{g}",
                                    name=f"sT{g}") for g in range(8)]
                    for qt in range(8):
                        s_in = sin_p.tile([128, SV], F32, tag="sin",
                                          name="sin")
                        nc.sync.dma_start(
                            out=s_in, in_=s_flat[qt * 128:(qt + 1) * 128, :])
                        for g in range(8):
                            ps = psum_tr.tile([128, 128], F32, tag="pstr",
                                              name="pstr")
                            nc.tensor.transpose(
                                ps, s_in[:, g * 128:(g + 1) * 128], ident)
                            nc.scalar.copy(
                                out=sT[g][:, qt * 128:(qt + 1) * 128], in_=ps)
                    srhs = lambda g, nh: sT[g][:, nh * 512:(nh + 1) * 512]

                for i in range(8):          # ph chunk (output partition)
                    for nh in range(2):     # q halves (N=512)
                        pm = psum_mm.tile([128, 512], F32, tag="psmm",
                                          name="psmm")
                        for g in range(8):  # contraction over sv
                            nc.tensor.matmul(
                                pm,
                                lhsT=wphi[g][:, i * 128:(i + 1) * 128],
                                rhs=srhs(g, nh),
                                start=(g == 0), stop=(g == 7))
                        # evacuate + per-partition b_phi bias add
                        nc.vector.tensor_scalar_add(
                            out=mT[i][:, nh * 512:(nh + 1) * 512],
                            in0=pm, scalar1=bphi_sb[:, i:i + 1])

            for ch in range(16):
                nc.gpsimd.dma_start(
                    out=wred[ch], in_=wred_d[ch * 128:(ch + 1) * 128, :])

            # --- A3/A4 per local batch: h(bf16), h.T via xbar, nT, e ---
            with ExitStack() as ph3:
                hT_p = ph3.enter_context(tc.tile_pool(name="hT", bufs=1))
                nT_p = ph3.enter_context(tc.tile_pool(name="nT", bufs=2))

                for b in range(BL):
                    # h -> bf16 by SWDGE cast-DMA (no f32 staging); the
                    # d-major transpose comes from the DMA xbar, replacing
                    # 128 PE transposes + ACT evacuations.
                    hT_bf = hT_p.tile([128, 4, TK], BF16, tag="hTbf",
                                      name="hTbf")
                    for kt in range(8):
                        nc.gpsimd.dma_start(
                            out=hbf[b][:, kt, :],
                            in_=h_d[b, kt * 128:(kt + 1) * 128, :])
                        # hT_bf[dp, dc, kt*128+kk] = h[b, kt*128+kk, dc*128+dp]
                        nc.sync.dma_start(
                            out=hT_bf[:, :, kt * 128:(kt + 1) * 128],
                            in_=hbf[b][:, kt, :], transpose=True)

                    nT = []
                    for pc in range(2):
                        t = nT_p.tile([128, TK], FCH, tag=f"nT{pc}",
                                      name=f"nT{pc}")
                        for nh in range(2):
                            pm = psum_mm.tile([128, 512], F32, tag="psmm",
                                              name="psmm")
                            for dj in range(4):
                                nc.tensor.matmul(
                                    pm,
                                    lhsT=wpsi[dj][:, pc * 128:(pc + 1) * 128],
                                    rhs=hT_bf[:, dj, nh * 512:(nh + 1) * 512],
                                    start=(dj == 0), stop=(dj == 3))
                            nc.scalar.copy(
                                out=t[:, nh * 512:(nh + 1) * 512], in_=pm)
                        nT.append(t)

                    # mask bias tiles for this batch: (mask-1)*BIG, per qc
                    mbias = []
                    for qc in range(2):
                        mt = mask_p.tile([128, TK], F32, tag="mask",
                                         name="mask")
                        nc.sync.dma_start(
                            out=mt,
                            in_=mask_d[b * TQ + qc * 128:
                                       b * TQ + qc * 128 + 128, :])
                        mb = mb_p.tile([128, TK], F32, tag="mb", name="mb")
                        nc.vector.tensor_scalar(
                            out=mb, in0=mt, scalar1=MASK_BIG,
                            scalar2=-MASK_BIG, op0=ALU.mult, op1=ALU.add)
                        mbias.append(mb)

                    # e for this batch: [q,k] per (head, q-chunk)
                    for hh in range(H):
                        for qc in range(2):
                            pe_ps = psum_e.tile([128, TK], F32, tag="pse",
                                                name="pse")
                            for nh in range(2):
                                for pc in range(2):
                                    lhs = mT[2 * hh + pc][
                                        :, b * TQ + qc * 128:
                                        b * TQ + qc * 128 + 128]
                                    nc.tensor.matmul(
                                        pe_ps[:, nh * 512:(nh + 1) * 512],
                                        lhsT=_r(lhs),
                                        rhs=_r(nT[pc][:, nh * 512:(nh + 1) * 512]),
                                        start=(pc == 0), stop=(pc == 1))
                            # raw-e per-row max straight off PSUM
                            lm = lm_p.tile([128, 1], F32, tag="lm", name="lm")
                            nc.vector.tensor_reduce(
                                out=lm, in_=pe_ps, axis=AX.X, op=ALU.max)
                            nc.vector.tensor_tensor(
                                out=locmax[:, hh:hh + 1],
                                in0=locmax[:, hh:hh + 1], in1=lm, op=ALU.max)
                            # evacuate with additive mask fold
                            e_sb = esb_p.tile([128, TK], F32, tag="esb",
                                              name="esb")
                            nc.vector.scalar_tensor_tensor(
                                out=e_sb, in0=pe_ps, scalar=1.0,
                                in1=mbias[qc], op0=ALU.mult, op1=ALU.add)
                            nc.gpsimd.dma_start(
                                out=e_spill[b, hh, qc], in_=e_sb)

            # --- A5: global max via AllReduce(max) ---
            ps = psum_tr.tile([128, 128], F32, tag="pstr", name="pstr")
            nc.tensor.transpose(ps[:H, :], locmax, ident)
            nc.vector.tensor_reduce(
                out=gmax_col[:H, :], in_=ps[:H, :], axis=AX.X, op=ALU.max)
            nc.sync.dma_start(out=cc_in[:], in_=gmax_col[:H, :])
            nc.gpsimd.collective_compute(
                "AllReduce", ALU.max,
                replica_groups=[list(range(NCORES))],
                ins=[cc_in[:]], outs=[cc_out[:]])
            nc.sync.dma_start(out=emax_b,
                              in_=_ap(cc_out[:], [[0, 128], [1, H]]))
            nc.scalar.mul(out=negmax, in_=emax_b, mul=-1.0)

        # ---------------- Phase C ----------------
        with ExitStack() as phc:
            bred_bc = phc.enter_context(
                tc.tile_pool(name="bredp", bufs=1)).tile(
                    [128, LV], F32, tag="bred", name="bred")
            nc.sync.dma_start(out=bred_bc,
                              in_=_ap(bred_d[:], [[0, 128], [1, LV]]))

            ein_p = phc.enter_context(tc.tile_pool(name="ein", bufs=8))
            exps_p = phc.enter_context(tc.tile_pool(name="exps", bufs=3))
            af_p = phc.enter_context(tc.tile_pool(name="af", bufs=3))
            abf_p = phc.enter_context(tc.tile_pool(name="abf", bufs=3))
            aT_p = phc.enter_context(tc.tile_pool(name="aT", bufs=2))
            ctxT_p = phc.enter_context(tc.tile_pool(name="ctxT", bufs=2))
            csb_p = phc.enter_context(tc.tile_pool(name="csb", bufs=2))
            sm_p = phc.enter_context(tc.tile_pool(name="smallp", bufs=8))
            psum_ctx = phc.enter_context(
                tc.tile_pool(name="psum_ctx", bufs=3, space="PSUM"))
            psum_c = phc.enter_context(
                tc.tile_pool(name="psum_c", bufs=2, space="PSUM"))

            for b in range(BL):
                # softmax + a output + bf16 transpose, per (head, q-chunk)
                aT = aT_p.tile([128, H, 2, 8, 128], BF16, tag="aT", name="aT")
                for hh in range(H):
                    for qc in range(2):
                        e_in = ein_p.tile([128, TK], F32, tag="ein",
                                          name="ein")
                        nc.sync.dma_start(out=e_in, in_=e_spill[b, hh, qc])
                        exps = exps_p.tile([128, TK], F32, tag="exps",
                                           name="exps")
                        rowsum = sm_p.tile([128, 1], F32, tag="rowsum",
                                           name="rowsum")
                        # exps = exp(e_masked - emax); rowsum = sum_k exps
                        nc.scalar.activation(
                            exps, e_in, ACTF.Exp,
                            bias=negmax[:, hh:hh + 1], scale=1.0,
                            accum_out=rowsum)
                        recip = sm_p.tile([128, 1], F32, tag="recip",
                                          name="recip")
                        nc.vector.tensor_scalar_add(
                            out=recip, in0=rowsum, scalar1=EPS)
                        nc.vector.reciprocal(out=recip, in_=recip)
                        a_f = af_p.tile([128, TK], F32, tag="af", name="af")
                        nc.vector.tensor_scalar_mul(
                            out=a_f, in0=exps, scalar1=recip)
                        nc.gpsimd.dma_start(
                            out=a_d[b, hh, qc * 128:qc * 128 + 128, :],
                            in_=a_f)
                        a_bf = abf_p.tile([128, TK], BF16, tag="abf",
                                          name="abf")
                        nc.scalar.copy(out=a_bf, in_=a_f)  # f32 -> bf16
                        if AT_MODE == "xbar3d":
                            # one xbar call: out[kp, kc, q] = in[q, kc*128+kp]
                            nc.sync.dma_start(
                                out=aT[:, hh, qc, :, :], in_=a_bf,
                                transpose=True)
                        else:  # PE transpose fallback
                            for kc in range(8):
                                pst = psum_ctx.tile(
                                    [128, 512], BF16, tag="psat", name="psat")
                                nc.tensor.transpose(
                                    pst[:, :128],
                                    a_bf[:, kc * 128:(kc + 1) * 128],
                                    ident_bf)
                                nc.vector.tensor_copy(
                                    out=aT[:, hh, qc, kc, :],
                                    in_=pst[:, :128])

                # ctxT[d, q] per head-pair (N = 2 heads * 256 q = 512)
                ctxT = [ctxT_p.tile([128, TQ], BF16, tag=f"ctxT{ch}",
                                    name=f"ctxT{ch}")
                        for ch in range(16)]
                for hp in range(2):
                    for dj in range(4):
                        pm = psum_ctx.tile([128, 512], F32, tag="psctx",
                                           name="psctx")
                        for kc in range(8):
                            nc.tensor.matmul(
                                pm,
                                lhsT=hbf[b][:, kc, dj * 128:(dj + 1) * 128],
                                rhs=aT[:, hp * 2:hp * 2 + 2, :, kc, :],
                                start=(kc == 0), stop=(kc == 7))
                        for hpos in range(2):
                            hh = hp * 2 + hpos
                            nc.scalar.copy(
                                out=ctxT[hh * 4 + dj],
                                in_=pm[:, hpos * 256:(hpos + 1) * 256])

                # c = ctxT.T @ W_red + b_red
                for qs in range(2):
                    pcs = psum_c.tile([128, 512], F32, tag="psc", name="psc")
                    for ch in range(16):
                        nc.tensor.matmul(
                            pcs,
                            lhsT=ctxT[ch][:, qs * 128:(qs + 1) * 128],
                            rhs=wred[ch],
                            start=(ch == 0), stop=(ch == 15))
                    c_sb = csb_p.tile([128, 512], F32, tag="csb", name="csb")
                    nc.vector.scalar_tensor_tensor(
                        out=c_sb, in0=pcs, scalar=1.0, in1=bred_bc,
                        op0=ALU.mult, op1=ALU.add)
                    nc.gpsimd.dma_start(
                        out=c_d[b * TQ + qs * 128: b * TQ + qs * 128 + 128, :],
                        in_=c_sb)

    nc.finalize()
    return nc


_CACHE = {}


def kernel(**inputs):
    s = np.ascontiguousarray(np.asarray(inputs["s"], dtype=np.float32))
    h = np.ascontiguousarray(np.asarray(inputs["h"], dtype=np.float32))
    len_mask = np.ascontiguousarray(
        np.asarray(inputs["len_mask"], dtype=np.float32))
    W_phi = np.ascontiguousarray(np.asarray(inputs["W_phi"], dtype=np.float32))
    b_phi = np.ascontiguousarray(np.asarray(inputs["b_phi"], dtype=np.float32))
    W_psi = np.ascontiguousarray(np.asarray(inputs["W_psi"], dtype=np.float32))
    W_red = np.ascontiguousarray(np.asarray(inputs["W_red"], dtype=np.float32))
    b_red = np.ascontiguousarray(np.asarray(inputs["b_red"], dtype=np.float32))

    if "nc" not in _CACHE:
        _CACHE["nc"] = build_kernel()
    nc = _CACHE["nc"]

    in_maps = []
    for core in range(NCORES):
        b0 = core * BL
        in_maps.append({
            "s": s[b0:b0 + BL],
            "h": h[b0:b0 + BL],
            "len_mask": len_mask[b0 * TQ:(b0 + BL) * TQ],
            "W_phi": W_phi, "b_phi": b_phi, "W_psi": W_psi,
            "W_red": W_red, "b_red": b_red,
        })

    res = run_bass_kernel_spmd(nc, in_maps, core_ids=list(range(NCORES)))
    results = res.results

    c = np.concatenate(
        [results[i]["c_out"].reshape(BL, TQ, LV) for i in range(NCORES)],
        axis=0)
    a = np.concatenate(
        [results[i]["a_out"] for i in range(NCORES)], axis=0)
    return c.astype(np.float32), a.astype(np.float32)


if __name__ == "__main__":
    rng = np.random.default_rng(0)
    ins = {
        "s": rng.standard_normal((B_FULL, TQ, SV), dtype=np.float32),
        "h": rng.standard_normal((B_FULL, TK, LV), dtype=np.float32),
        "len_mask": (rng.random((B_FULL * TQ, TK)) < 0.9).astype(np.float32),
        "W_phi": rng.standard_normal((SV, P * H), dtype=np.float32) * 0.02,
        "b_phi": np.zeros((P * H,), np.float32),
        "W_psi": rng.standard_normal((LV, P), dtype=np.float32) * 0.02,
        "W_red": rng.standard_normal((LV * H, LV), dtype=np.float32) * 0.02,
        "b_red": np.zeros((LV,), np.float32),
    }
    c, a = kernel(**ins)
    print("c", c.shape, c.dtype, "a", a.shape, a.dtype)


# revision 17
# speedup vs baseline: 3.6570x; 3.6570x over previous
"""Distributed Trainium2 kernel for the sparse-attention nn.Module.

Reference computation (per full batch B=32):
    m  = s @ W_phi + b_phi                    [B, Tq, H*P]
    n  = h @ W_psi                            [B, Tk, P]
    e  = einsum('bqhp,bkp->hbqk', m.resh, n)  [H, B, Tq, Tk]
    emax = global per-head max of e           (max over B, Tq, Tk!)
    exps = exp(e - emax) * mask
    a  = exps / (sum_k exps + EPS)            [H, B, Tq, Tk]
    ctx = einsum('hbqk,bkd->bqhd', a, h)      -> [B, Tq, H*LV]
    c  = ctx @ W_red + b_red                  [B, Tq, LV]
    returns (c, a.transpose(1,0,2,3))

Sharding: data-parallel over batch across 8 NeuronCores (4 batches/core);
the only cross-core dependency is the global per-head max -> tiny
AllReduce(max) of 4 floats mid-kernel.

Precision: the e einsum accumulates in fp32 PSUM from fp32 mT/nT
tiles; the s/h projections and the attention-application side run bf16
(fp32 PSUM accumulation everywhere).  Exp amplifies score errors
(rowsums ~1e-4 vs EPS=1e-5), so full-bf16 scores would be risky; this
mix measures ~8.7e-3 rel err on HW vs the 2e-2 gate.  K_SBF=0 /
K_AT_MODE=pe env flags fall back to more conservative variants.

Layout strategy (per core, B_loc=4, Q=1024 query rows):
  - mT[ph, q] = W_phi.T @ s.T; s.T via PE-transpose.
  - nT[p, k]  = W_psi.T @ h.T; h.T via PE-transpose.
  - e[q, k] = matmul(lhsT=mT chunks, rhs=nT) fp32 -> PSUM; evacuated by a
    DVE scalar_tensor_tensor that also folds the mask in additively
    (e + (mask-1)*BIG), and spilled to DRAM (16 MiB of scores don't fit
    in SBUF).  Per-row max comes from a DVE reduce directly on the PSUM
    tile (raw e, pre-mask, matching the reference).
  - softmax: ACT exp with per-partition bias (-emax) and fused row-sum
    accumulation; DVE reciprocal + scale.
  - a (f32) is DMA'd out; a bf16 copy is transposed k-major via the DMA
    xbar (one 3D-out call per [128,1024] tile).
  - ctxT[d, q] = matmul(lhsT=h(bf16), rhs=aT(bf16)); two heads packed in
    the moving operand (N=512).
  - c[q, dout] = matmul(lhsT=ctxT chunks, rhs=W_red(bf16)) + b_red.

NB: tensor_tensor_reduce is avoided entirely -- it crashes the NC
(NRT_EXEC_UNIT_UNRECOVERABLE) on this runtime, from both PSUM and SBUF.
"""

import os as _os
import numpy as np
from contextlib import ExitStack

import concourse.bass as bass
import concourse.mybir as mybir
import concourse.tile as tile
from concourse import bacc
from concourse.bass_utils import run_bass_kernel_spmd
from concourse.masks import make_identity

F32 = mybir.dt.float32
F32R = mybir.dt.float32r
BF16 = mybir.dt.bfloat16
AX = mybir.AxisListType
ALU = mybir.AluOpType
ACTF = mybir.ActivationFunctionType

B_FULL, TQ, TK = 32, 256, 1024
SV, LV, P, H = 1024, 512, 256, 4
NCORES = 8
BL = B_FULL // NCORES      # local batches per core
Q = BL * TQ                # local query rows (1024)
EPS = 1e-5
NEG_INF = -1e30
MASK_BIG = 1e4             # additive mask: e + (mask-1)*MASK_BIG

AT_MODE = _os.environ.get("K_AT_MODE", "xbar3d")   # "xbar3d" | "pe"
USE_F32R = _os.environ.get("K_F32R", "1") == "1"   # float32r e-chain matmuls
TRACE_SIM = _os.environ.get("K_TRACE_SIM", "0") == "1"
USE_SBF = _os.environ.get("K_SBF", "1") == "1"   # bf16 s->m chain (xbar sT)


FCH = F32R if USE_F32R else F32   # dtype of score-chain matmul operands


def _r(ap):
    return ap


def _ap(handle_ap, ap_list, offset=None):
    """Build a raw AP (for broadcast / strided access) on a tensor."""
    return bass.AP(
        tensor=handle_ap.tensor,
        offset=handle_ap.offset if offset is None else offset,
        ap=ap_list,
    )


def build_kernel() -> bass.Bass:
    nc = bacc.Bacc()

    s_d = nc.declare_dram_parameter("s", [BL, TQ, SV], F32, isOutput=False)
    h_d = nc.declare_dram_parameter("h", [BL, TK, LV], F32, isOutput=False)
    mask_d = nc.declare_dram_parameter("len_mask", [Q, TK], F32, isOutput=False)
    wphi_d = nc.declare_dram_parameter("W_phi", [SV, P * H], F32, isOutput=False)
    bphi_d = nc.declare_dram_parameter("b_phi", [P * H], F32, isOutput=False)
    wpsi_d = nc.declare_dram_parameter("W_psi", [LV, P], F32, isOutput=False)
    wred_d = nc.declare_dram_parameter("W_red", [LV * H, LV], F32, isOutput=False)
    bred_d = nc.declare_dram_parameter("b_red", [LV], F32, isOutput=False)
    c_d = nc.declare_dram_parameter("c_out", [Q, LV], F32, isOutput=True)
    a_d = nc.declare_dram_parameter("a_out", [BL, H, TQ, TK], F32, isOutput=True)

    s_flat = s_d[:].rearrange("b q v -> (b q) v")      # [1024, 1024]

    e_spill = nc.dram_tensor("e_spill", [BL, H, 2, 128, TK], F32)
    cc_in = nc.dram_tensor("cc_in", [H], F32)
    cc_out = nc.dram_tensor("cc_out", [H], F32, addr_space="Shared")

    with ExitStack() as top:
        tc = top.enter_context(tile.TileContext(nc, trace_sim=TRACE_SIM))

        const = top.enter_context(tc.tile_pool(name="const", bufs=1))
        ident = const.tile([128, 128], F32, tag="ident", name="ident")
        make_identity(nc, ident)
        ident_bf = const.tile([128, 128], BF16, tag="identbf", name="identbf")
        nc.vector.tensor_copy(out=ident_bf, in_=ident)
        # b_phi laid out chunk-major: bphi_sb[p, i] = b_phi[i*128 + p].
        # Load as [8,128] rows (contiguous) then PE-transpose: a direct
        # [[1,128],[128,8]] DMA is 4-byte-strided and costs ~34us.
        bphi_sb = const.tile([128, 8], F32, tag="bphi", name="bphi")
        bphi_row = const.tile([128, 128], F32, tag="bphirow", name="bphirow")
        nc.gpsimd.dma_start(out=bphi_row[:8, :],
                            in_=_ap(bphi_d[:], [[128, 8], [1, 128]]))
        locmax = const.tile([128, H], F32, tag="locmax", name="locmax")
        nc.vector.memset(locmax, NEG_INF)
        gmax_col = const.tile([128, 1], F32, tag="gmaxcol", name="gmaxcol")
        emax_b = const.tile([128, H], F32, tag="emaxb", name="emaxb")
        negmax = const.tile([128, H], F32, tag="negmax", name="negmax")

        # W_red tiles allocated up front; their cast-DMAs are issued after
        # the s/W_phi casts (otherwise they either head the Pool queue and
        # delay the first matmuls, or sit at the barrier stalling a-stores)
        wred_p = top.enter_context(tc.tile_pool(name="wred", bufs=1))
        wred = [wred_p.tile([128, LV], BF16, tag=f"wred{ch}",
                            name=f"wred{ch}") for ch in range(16)]
        # hbf persists into phase C
        hbf_p = top.enter_context(tc.tile_pool(name="hbf", bufs=1))
        hbf = [hbf_p.tile([128, 8, LV], BF16, tag=f"hbf{b}", name=f"hbf{b}")
               for b in range(BL)]

        # ---------------- Phase A ----------------
        with ExitStack() as pha:
            psum_tr = pha.enter_context(
                tc.tile_pool(name="psum_tr", bufs=2, space="PSUM"))
            psum_mm = pha.enter_context(
                tc.tile_pool(name="psum_mm", bufs=2, space="PSUM"))
            psum_e = pha.enter_context(
                tc.tile_pool(name="psum_e", bufs=2, space="PSUM"))
            ps_b = psum_tr.tile([128, 128], F32, tag="pstr", name="pstr")
            nc.tensor.transpose(ps_b[:, :8], bphi_row[:8, :], ident[:8, :8])
            nc.scalar.copy(out=bphi_sb, in_=ps_b[:, :8])
            esb_p = pha.enter_context(tc.tile_pool(name="esb", bufs=4))
            lm_p = pha.enter_context(tc.tile_pool(name="lm", bufs=4))
            wpsi_p = pha.enter_context(tc.tile_pool(name="wpsi", bufs=1))
            wpsi = []
            for dj in range(4):
                t = wpsi_p.tile([128, P], BF16, tag=f"wpsi{dj}",
                                name=f"wpsi{dj}")
                nc.gpsimd.dma_start(
                    out=t, in_=wpsi_d[dj * 128:(dj + 1) * 128, :])
                wpsi.append(t)
            # mT persists through e (phase A only)
            mT_p = pha.enter_context(tc.tile_pool(name="mT", bufs=1))
            mT = [mT_p.tile([128, Q], FCH, tag=f"mT{i}", name=f"mT{i}")
                  for i in range(8)]

            # --- A1/A2: W_phi, s.T, mT = W_phi.T @ s.T (+ b_phi) ---
            with ExitStack() as ph2:
                sdt = BF16 if USE_SBF else FCH
                wphi_p = ph2.enter_context(tc.tile_pool(name="wphi", bufs=1))
                wphi = []
                sin_p = ph2.enter_context(tc.tile_pool(name="sin", bufs=3))
                if USE_SBF:
                    # s -> bf16 cast-DMA, transposed via the DMA xbar.
                    # W_phi casts are interleaved with the s casts so the
                    # first mT matmul isn't queued behind all 8 of them.
                    sT_p = ph2.enter_context(tc.tile_pool(name="sTb", bufs=1))
                    sT_bf = sT_p.tile([128, 8, Q], BF16, tag="sTb",
                                      name="sTb")
                    for qt in range(8):
                        s_in = sin_p.tile([128, SV], BF16, tag="sin",
                                          name="sin")
                        nc.gpsimd.dma_start(
                            out=s_in, in_=s_flat[qt * 128:(qt + 1) * 128, :])
                        nc.sync.dma_start(
                            out=sT_bf[:, :, qt * 128:(qt + 1) * 128],
                            in_=s_in, transpose=True)
                        t = wphi_p.tile([128, P * H], sdt, tag=f"wphi{qt}",
                                        name=f"wphi{qt}")
                        nc.gpsimd.dma_start(
                            out=t, in_=wphi_d[qt * 128:(qt + 1) * 128, :])
                        wphi.append(t)
                    srhs = lambda g, nh: sT_bf[:, g, nh * 512:(nh + 1) * 512]
                else:
                    for g in range(8):
                        t = wphi_p.tile([128, P * H], sdt, tag=f"wphi{g}",
                                        name=f"wphi{g}")
                        nc.sync.dma_start(
                            out=t.bitcast(F32),
                            in_=wphi_d[g * 128:(g + 1) * 128, :])
                        wphi.append(t)
                    sT_p = ph2.enter_context(tc.tile_pool(name="sT", bufs=1))
                    sT = [sT_p.tile([128, Q], FCH, tag=f"sT{g}",
                                    name=f"sT{g}") for g in range(8)]
                    for qt in range(8):
                        s_in = sin_p.tile([128, SV], F32, tag="sin",
                                          name="sin")
                        nc.sync.dma_start(
                            out=s_in, in_=s_flat[qt * 128:(qt + 1) * 128, :])
                        for g in range(8):
                            ps = psum_tr.tile([128, 128], F32, tag="pstr",
                                              name="pstr")
                            nc.tensor.transpose(
                                ps, s_in[:, g * 128:(g + 1) * 128], ident)
                            nc.scalar.copy(
                                out=sT[g][:, qt * 128:(qt + 1) * 128], in_=ps)
                    srhs = lambda g, nh: sT[g][:, nh * 512:(nh + 1) * 512]

                for i in range(8):          # ph chunk (output partition)
                    for nh in range(2):     # q halves (N=512)
                        pm = psum_mm.tile([128, 512], F32, tag="psmm",
                                          name="psmm")
                        for g in range(8):  # contraction over sv
                            nc.tensor.matmul(
                                pm,
                                lhsT=wphi[g][:, i * 128:(i + 1) * 128],
                                rhs=srhs(g, nh),
                                start=(g == 0), stop=(g == 7))
                        # evacuate + per-partition b_phi bias add
                        nc.vector.tensor_scalar_add(
                            out=mT[i][:, nh * 512:(nh + 1) * 512],
                            in0=pm, scalar1=bphi_sb[:, i:i + 1])

            for ch in range(16):
                nc.gpsimd.dma_start(
                    out=wred[ch], in_=wred_d[ch * 128:(ch + 1) * 128, :])

            # --- A3/A4 per local batch: h(bf16), h.T via xbar, nT, e ---
            with ExitStack() as ph3:
                hT_p = ph3.enter_context(tc.tile_pool(name="hT", bufs=1))
                nT_p = ph3.enter_context(tc.tile_pool(name="nT", bufs=2))

                for b in range(BL):
                    # h -> bf16 by SWDGE cast-DMA (no f32 staging); the
                    # d-major transpose comes from the DMA xbar, replacing
                    # 128 PE transposes + ACT evacuations.
                    hT_bf = hT_p.tile([128, 4, TK], BF16, tag="hTbf",
                                      name="hTbf")
                    for kt in range(8):
                        nc.gpsimd.dma_start(
                            out=hbf[b][:, kt, :],
                            in_=h_d[b, kt * 128:(kt + 1) * 128, :])
                        # hT_bf[dp, dc, kt*128+kk] = h[b, kt*128+kk, dc*128+dp]
                        nc.sync.dma_start(
                            out=hT_bf[:, :, kt * 128:(kt + 1) * 128],
                            in_=hbf[b][:, kt, :], transpose=True)

                    nT = []
                    for pc in range(2):
                        t = nT_p.tile([128, TK], FCH, tag=f"nT{pc}",
                                      name=f"nT{pc}")
                        for nh in range(2):
                            pm = psum_mm.tile([128, 512], F32, tag="psmm",
                                              name="psmm")
                            for dj in range(4):
                                nc.tensor.matmul(
                                    pm,
                                    lhsT=wpsi[dj][:, pc * 128:(pc + 1) * 128],
                                    rhs=hT_bf[:, dj, nh * 512:(nh + 1) * 512],
                                    start=(dj == 0), stop=(dj == 3))
                            nc.scalar.copy(
                                out=t[:, nh * 512:(nh + 1) * 512], in_=pm)
                        nT.append(t)

                    # e for this batch: [q,k] per (head, q-chunk)
                    for hh in range(H):
                        for qc in range(2):
                            pe_ps = psum_e.tile([128, TK], F32, tag="pse",
                                                name="pse")
                            for nh in range(2):
                                for pc in range(2):
                                    lhs = mT[2 * hh + pc][
                                        :, b * TQ + qc * 128:
                                        b * TQ + qc * 128 + 128]
                                    nc.tensor.matmul(
                                        pe_ps[:, nh * 512:(nh + 1) * 512],
                                        lhsT=_r(lhs),
                                        rhs=_r(nT[pc][:, nh * 512:(nh + 1) * 512]),
                                        start=(pc == 0), stop=(pc == 1))
                            # raw-e per-row max straight off PSUM
                            lm = lm_p.tile([128, 1], F32, tag="lm", name="lm")
                            nc.vector.tensor_reduce(
                                out=lm, in_=pe_ps, axis=AX.X, op=ALU.max)
                            nc.vector.tensor_tensor(
                                out=locmax[:, hh:hh + 1],
                                in0=locmax[:, hh:hh + 1], in1=lm, op=ALU.max)
                            # evacuate raw e on ACT (mask is folded in
                            # phase C); keeps the e pipeline off DVE
                            e_sb = esb_p.tile([128, TK], F32, tag="esb",
                                              name="esb")
                            nc.scalar.copy(out=e_sb, in_=pe_ps)
                            nc.gpsimd.dma_start(
                                out=e_spill[b, hh, qc], in_=e_sb)

            # --- A5: global max via AllReduce(max) ---
            ps = psum_tr.tile([128, 128], F32, tag="pstr", name="pstr")
            nc.tensor.transpose(ps[:H, :], locmax, ident)
            nc.vector.tensor_reduce(
                out=gmax_col[:H, :], in_=ps[:H, :], axis=AX.X, op=ALU.max)
            nc.sync.dma_start(out=cc_in[:], in_=gmax_col[:H, :])
            nc.gpsimd.collective_compute(
                "AllReduce", ALU.max,
                replica_groups=[list(range(NCORES))],
                ins=[cc_in[:]], outs=[cc_out[:]])
            nc.sync.dma_start(out=emax_b,
                              in_=_ap(cc_out[:], [[0, 128], [1, H]]))
            nc.scalar.mul(out=negmax, in_=emax_b, mul=-1.0)

        # ---------------- Phase C ----------------
        with ExitStack() as phc:
            bred_bc = phc.enter_context(
                tc.tile_pool(name="bredp", bufs=1)).tile(
                    [128, LV], F32, tag="bred", name="bred")
            nc.sync.dma_start(out=bred_bc,
                              in_=_ap(bred_d[:], [[0, 128], [1, LV]]))

            ein_p = phc.enter_context(tc.tile_pool(name="ein", bufs=8))
            mask_p = phc.enter_context(tc.tile_pool(name="maskp", bufs=2))
            mb_p = phc.enter_context(tc.tile_pool(name="mbp", bufs=3))
            exps_p = phc.enter_context(tc.tile_pool(name="exps", bufs=3))
            af_p = phc.enter_context(tc.tile_pool(name="af", bufs=3))
            abf_p = phc.enter_context(tc.tile_pool(name="abf", bufs=3))
            aT_p = phc.enter_context(tc.tile_pool(name="aT", bufs=2))
            ctxT_p = phc.enter_context(tc.tile_pool(name="ctxT", bufs=2))
            csb_p = phc.enter_context(tc.tile_pool(name="csb", bufs=2))
            sm_p = phc.enter_context(tc.tile_pool(name="smallp", bufs=8))
            psum_ctx = phc.enter_context(
                tc.tile_pool(name="psum_ctx", bufs=3, space="PSUM"))
            psum_c = phc.enter_context(
                tc.tile_pool(name="psum_c", bufs=2, space="PSUM"))

            for b in range(BL):
                # mask bias tiles for this batch: (mask-1)*BIG, per qc
                mbias = []
                for qc in range(2):
                    mt = mask_p.tile([128, TK], F32, tag="mask", name="mask")
                    nc.sync.dma_start(
                        out=mt,
                        in_=mask_d[b * TQ + qc * 128:
                                   b * TQ + qc * 128 + 128, :])
                    mb = mb_p.tile([128, TK], F32, tag="mb", name="mb")
                    nc.vector.tensor_scalar(
                        out=mb, in0=mt, scalar1=MASK_BIG,
                        scalar2=-MASK_BIG, op0=ALU.mult, op1=ALU.add)
                    mbias.append(mb)
                # softmax + a output + bf16 transpose, per (head, q-chunk)
                aT = aT_p.tile([128, H, 2, 8, 128], BF16, tag="aT", name="aT")
                for hh in range(H):
                    for qc in range(2):
                        e_in = ein_p.tile([128, TK], F32, tag="ein",
                                          name="ein")
                        nc.sync.dma_start(out=e_in, in_=e_spill[b, hh, qc])
                        # fold mask additively (collective-independent, so
                        # it overlaps the AllReduce)
                        nc.vector.tensor_tensor(
                            out=e_in, in0=e_in, in1=mbias[qc], op=ALU.add)
                        exps = exps_p.tile([128, TK], F32, tag="exps",
                                           name="exps")
                        rowsum = sm_p.tile([128, 1], F32, tag="rowsum",
                                           name="rowsum")
                        # exps = exp(e_masked - emax); rowsum = sum_k exps
                        nc.scalar.activation(
                            exps, e_in, ACTF.Exp,
                            bias=negmax[:, hh:hh + 1], scale=1.0,
                            accum_out=rowsum)
                        recip = sm_p.tile([128, 1], F32, tag="recip",
                                          name="recip")
                        nc.vector.tensor_scalar_add(
                            out=recip, in0=rowsum, scalar1=EPS)
                        nc.vector.reciprocal(out=recip, in_=recip)
                        a_f = af_p.tile([128, TK], F32, tag="af", name="af")
                        nc.vector.tensor_scalar_mul(
                            out=a_f, in0=exps, scalar1=recip)
                        nc.gpsimd.dma_start(
                            out=a_d[b, hh, qc * 128:qc * 128 + 128, :],
                            in_=a_f)
                        a_bf = abf_p.tile([128, TK], BF16, tag="abf",
                                          name="abf")
                        nc.scalar.copy(out=a_bf, in_=a_f)  # f32 -> bf16
                        if AT_MODE == "xbar3d":
                            # one xbar call: out[kp, kc, q] = in[q, kc*128+kp]
                            nc.sync.dma_start(
                                out=aT[:, hh, qc, :, :], in_=a_bf,
                                transpose=True)
                        else:  # PE transpose fallback
                            for kc in range(8):
                                pst = psum_ctx.tile(
                                    [128, 512], BF16, tag="psat", name="psat")
                                nc.tensor.transpose(
                                    pst[:, :128],
                                    a_bf[:, kc * 128:(kc + 1) * 128],
                                    ident_bf)
                                nc.vector.tensor_copy(
                                    out=aT[:, hh, qc, kc, :],
                                    in_=pst[:, :128])

                # ctxT[d, q] per head-pair (N = 2 heads * 256 q = 512)
                ctxT = [ctxT_p.tile([128, TQ], BF16, tag=f"ctxT{ch}",
                                    name=f"ctxT{ch}")
                        for ch in range(16)]
                for hp in range(2):
                    for dj in range(4):
                        pm = psum_ctx.tile([128, 512], F32, tag="psctx",
                                           name="psctx")
                        for kc in range(8):
                            nc.tensor.matmul(
                                pm,
                                lhsT=hbf[b][:, kc, dj * 128:(dj + 1) * 128],
                                rhs=aT[:, hp * 2:hp * 2 + 2, :, kc, :],
                                start=(kc == 0), stop=(kc == 7))
                        for hpos in range(2):
                            hh = hp * 2 + hpos
                            nc.scalar.copy(
                                out=ctxT[hh * 4 + dj],
                                in_=pm[:, hpos * 256:(hpos + 1) * 256])

                # c = ctxT.T @ W_red + b_red
                for qs in range(2):
                    pcs = psum_c.tile([128, 512], F32, tag="psc", name="psc")
                    for ch in range(16):
                        nc.tensor.matmul(
                            pcs,
                            lhsT=ctxT[ch][:, qs * 128:(qs + 1) * 128],
                            rhs=wred[ch],
                            start=(ch == 0), stop=(ch == 15))
                    c_sb = csb_p.tile([128, 512], F32, tag="csb", name="csb")
                    nc.vector.scalar_tensor_tensor(
                        out=c_sb, in0=pcs, scalar=1.0, in1=bred_bc,
                        op0=ALU.mult, op1=ALU.add)
                    nc.gpsimd.dma_start(
                        out=c_d[b * TQ + qs * 128: b * TQ + qs * 128 + 128, :],
                        in_=c_sb)

    nc.finalize()
    return nc


_CACHE = {}


def kernel(**inputs):
    s = np.ascontiguousarray(np.asarray(inputs["s"], dtype=np.float32))
    h = np.ascontiguousarray(np.asarray(inputs["h"], dtype=np.float32))
    len_mask = np.ascontiguousarray(
        np.asarray(inputs["len_mask"], dtype=np.float32))
    W_phi = np.ascontiguousarray(np.asarray(inputs["W_phi"], dtype=np.float32))
    b_phi = np.ascontiguousarray(np.asarray(inputs["b_phi"], dtype=np.float32))
    W_psi = np.ascontiguousarray(np.asarray(inputs["W_psi"], dtype=np.float32))
    W_red = np.ascontiguousarray(np.asarray(inputs["W_red"], dtype=np.float32))
    b_red = np.ascontiguousarray(np.asarray(inputs["b_red"], dtype=np.float32))

    if "nc" not in _CACHE:
        _CACHE["nc"] = build_kernel()
    nc = _CACHE["nc"]

    in_maps = []
    for core in range(NCORES):
        b0 = core * BL
        in_maps.append({
            "s": s[b0:b0 + BL],
            "h": h[b0:b0 + BL],
            "len_mask": len_mask[b0 * TQ:(b0 + BL) * TQ],
            "W_phi": W_phi, "b_phi": b_phi, "W_psi": W_psi,
            "W_red": W_red, "b_red": b_red,
        })

    res = run_bass_kernel_spmd(nc, in_maps, core_ids=list(range(NCORES)))
    results = res.results

    c = np.concatenate(
        [results[i]["c_out"].reshape(BL, TQ, LV) for i in range(NCORES)],
        axis=0)
    a = np.concatenate(
        [results[i]["a_out"] for i in range(NCORES)], axis=0)
    return c.astype(np.float32), a.astype(np.float32)


if __name__ == "__main__":
    rng = np.random.default_rng(0)
    ins = {
        "s": rng.standard_normal((B_FULL, TQ, SV), dtype=np.float32),
        "h": rng.standard_normal((B_FULL, TK, LV), dtype=np.float32),
        "len_mask": (rng.random((B_FULL * TQ, TK)) < 0.9).astype(np.float32),
        "W_phi": rng.standard_normal((SV, P * H), dtype=np.float32) * 0.02,
        "b_phi": np.zeros((P * H,), np.float32),
        "W_psi": rng.standard_normal((LV, P), dtype=np.float32) * 0.02,
        "W_red": rng.standard_normal((LV * H, LV), dtype=np.float32) * 0.02,
        "b_red": np.zeros((LV,), np.float32),
    }
    c, a = kernel(**ins)
    print("c", c.shape, c.dtype, "a", a.shape, a.dtype)
